# revision 22
# baseline (speedup 1.0000x reference)
import sys

sys.path.insert(0, "/opt/trn_rl_repo")
import numpy as np
import concourse.bacc as bacc
import concourse.mybir as mybir
import concourse.tile as tile
from concourse.bass_utils import run_bass_kernel_spmd

F32R = mybir.dt.float32r
F32 = mybir.dt.float32
FP16 = mybir.dt.float16
AF = mybir.ActivationFunctionType

B, S, D, H, DV = 2, 2048, 1024, 16, 64
NKT = 8     # k-tiles of 128 over D
NJ = 4      # query chunks of 512
NB = 16     # key blocks of 128
HPC = 4     # heads per core

_NC = None


def _build():
    nc = bacc.Bacc(target_bir_lowering=False)
    xq = nc.dram_tensor("xq", [D, S], FP16, kind="ExternalInput")
    xk = nc.dram_tensor("xk", [D, S], FP16, kind="ExternalInput")
    xv = nc.dram_tensor("xv", [D, S], FP16, kind="ExternalInput")
    wq = nc.dram_tensor("wq", [D, 256], FP16, kind="ExternalInput")
    wk = nc.dram_tensor("wk", [D, 256], FP16, kind="ExternalInput")
    wv = nc.dram_tensor("wv", [D, 256], FP16, kind="ExternalInput")
    w0 = nc.dram_tensor("w0", [256, D], FP16, kind="ExternalInput")
    cmt = nc.dram_tensor("cmt", [128, 256], F32R, kind="ExternalInput")
    yt = nc.dram_tensor("yt", [D, S], FP16, kind="ExternalOutput")

    with tile.TileContext(nc) as tc:
        with tc.tile_pool(name="pp", bufs=1) as pp:
            # Per-head Q with the other head's 64 rows zeroed: scores use the
            # full dense 128-row kt block as stationary (full PE array) -- the
            # zero rows kill the other head's contribution.
            qtz = [pp.tile([128, S], FP16, name=f"qtz{h}", tag=f"qtz{h}")
                   for h in range(HPC)]
            kt = [pp.tile([128, S], FP16, name=f"kt{p}", tag=f"kt{p}") for p in range(2)]
            # V padded to 128 cols, all non-V columns = ones. Even heads keep
            # V in cols 0:64 (numerators -> out rows 0:64, den at row 64);
            # odd heads keep V in cols 64:128 (numerators -> out rows 64:128,
            # den read from row 0).
            v2 = pp.tile([128, NB, HPC, 128], F32R, name="v2", tag="v2")
            bcsel = pp.tile([128, 128], F32R, name="bcsel", tag="bcsel")
            w0p = [pp.tile([128, D], FP16, name=f"w0p{p}", tag=f"w0p{p}") for p in range(2)]
            ot2 = [pp.tile([128, S], FP16, name=f"ot2{p}", tag=f"ot2{p}") for p in range(2)]
            cm_sb = pp.tile([128, 256], F32R, name="cmsb", tag="cmsb")

            ones_stage = pp.tile([128, 512], F32, name="ones_stage", tag="ones_stage")
            nc.vector.memset(ones_stage[:, :], 1.0)
            for i in range(NB):
                nc.vector.tensor_copy(v2[:, i, :, :], ones_stage[:, :])
            nc.vector.tensor_copy(bcsel[64:65, :], ones_stage[64:65, 0:128])
            nc.vector.tensor_copy(bcsel[0:1, :], ones_stage[0:1, 0:128])
            for h in range(HPC):
                dead = 64 * (1 - (h % 2))
                nc.vector.memset(qtz[h][dead:dead + 64, :], 0.0)

            # xv/wv live through phase B (late V wavelets read them there)
            with tc.tile_pool(name="wtv", bufs=1) as wtv, \
                 tc.tile_pool(name="xvp", bufs=1) as xvp, \
                 tc.tile_pool(name="pb", bufs=1) as pb:
                wv_t = [wtv.tile([128, 256], FP16, name=f"wv{k}", tag=f"wv{k}")
                        for k in range(NKT)]
                xv_t = [xvp.tile([128, S], FP16, name=f"xv{k}", tag="xv", bufs=8)
                        for k in range(NKT)]
                for k in range(NKT):
                    nc.gpsimd.dma_start(out=wv_t[k][:, :],
                                        in_=wv[128 * k:128 * k + 128, :])
                for k in range(NKT):
                    eng = nc.sync if k % 2 == 0 else nc.gpsimd
                    eng.dma_start(out=xv_t[k][:, :], in_=xv[128 * k:128 * k + 128, :])

                # ---- Phase A: V first half, then Q, K projections ----
                with tc.tile_pool(name="wts", bufs=1) as wts, \
                     tc.tile_pool(name="xin", bufs=1) as xin, \
                     tc.tile_pool(name="psA", bufs=8, space="PSUM") as psA:
                    wq_t = [wts.tile([128, 256], FP16, name=f"wq{k}", tag=f"wq{k}")
                            for k in range(NKT)]
                    wk_t = [wts.tile([128, 256], FP16, name=f"wk{k}", tag=f"wk{k}")
                            for k in range(NKT)]
                    xq_t = [xin.tile([128, S], FP16, name=f"xq{k}", tag="xq", bufs=8)
                            for k in range(NKT)]
                    xk_t = [xin.tile([128, S], FP16, name=f"xk{k}", tag="xk", bufs=8)
                            for k in range(NKT)]
                    for k in range(NKT):
                        nc.scalar.dma_start(out=wq_t[k][:, :],
                                            in_=wq[128 * k:128 * k + 128, :])
                    for k in range(NKT):
                        eng = nc.sync if k % 2 == 0 else nc.scalar
                        eng.dma_start(out=xq_t[k][:, :], in_=xq[128 * k:128 * k + 128, :])
                    for k in range(NKT):
                        nc.scalar.dma_start(out=wk_t[k][:, :],
                                            in_=wk[128 * k:128 * k + 128, :])
                    for k in range(NKT):
                        eng = nc.sync if k % 2 == 0 else nc.scalar
                        eng.dma_start(out=xk_t[k][:, :], in_=xk[128 * k:128 * k + 128, :])
                    for p in range(2):
                        nc.sync.dma_start(out=w0p[p][:, :],
                                          in_=w0[128 * p:128 * p + 128, :])
                    nc.sync.dma_start(out=cm_sb[:, :], in_=cmt[:, :])

                    # V projection, first 8 st-groups (key blocks 0:8),
                    # k-outer so compute paces the xv DMA arrivals.
                    vps = [psA.tile([128, 4, 64], F32, name=f"vps{g}", tag="pj",
                                    bufs=8) for g in range(8)]
                    for k in range(NKT):
                        for g in range(8):
                            nc.tensor.matmul(
                                vps[g][:, :, :],
                                xv_t[k][:, 128 * g:128 * g + 128],
                                wv_t[k][:, :],
                                start=(k == 0), stop=(k == NKT - 1))
                    for g in range(8):
                        nc.vector.tensor_copy(v2[:, g, 0:4:2, 0:64],
                                              vps[g][:, 0:4:2, :])
                        nc.vector.tensor_copy(v2[:, g, 1:4:2, 64:128],
                                              vps[g][:, 1:4:2, :])

                    # Q then K: 8 psum groups [128,512] each, 512-wide
                    # matmuls, k-outer so compute paces DMA arrival.
                    for which, wt, xt in (("q", wq_t, xq_t), ("k", wk_t, xk_t)):
                        qps = [psA.tile([128, 512], F32, name=f"pj{which}{i}",
                                        tag="pj", bufs=8) for i in range(8)]
                        for k in range(NKT):
                            for p in range(2):
                                for jj in range(4):
                                    nc.tensor.matmul(
                                        qps[4 * p + jj][:, :],
                                        wt[k][:, 128 * p:128 * p + 128],
                                        xt[k][:, 512 * jj:512 * jj + 512],
                                        start=(k == 0), stop=(k == NKT - 1))
                        # q copies ride on DVE under the K matmul stream; the
                        # k copies gate the first scores, so they go jj-major
                        # and alternate DVE/ACT to land the jj=0 pair fastest.
                        if which == "q":
                            for p in range(2):
                                for jj in range(4):
                                    sl = slice(512 * jj, 512 * jj + 512)
                                    g = qps[4 * p + jj]
                                    nc.vector.tensor_copy(qtz[2 * p][0:64, sl],
                                                          g[0:64, :])
                                    nc.vector.tensor_copy(qtz[2 * p + 1][64:128, sl],
                                                          g[64:128, :])
                        else:
                            ci = 0
                            for jj in range(4):
                                for p in range(2):
                                    sl = slice(512 * jj, 512 * jj + 512)
                                    g = qps[4 * p + jj]
                                    if ci % 2 == 0:
                                        nc.vector.tensor_copy(kt[p][:, sl], g[:, :])
                                    else:
                                        nc.scalar.copy(kt[p][:, sl], g[:, :])
                                    ci += 1

                # ---- Phase B: attention + out-projection + late V half ----
                with tc.tile_pool(name="psB", bufs=1, space="PSUM") as psB:

                    def emit_v_wavelet(w):
                        # st-groups 8+2w, 9+2w; xv is fully resident by now.
                        vt = [psB.tile([128, 512], F32, name=f"vw{w}{gg}",
                                       tag="ypsv", bufs=2) for gg in range(2)]
                        vws = [t[:, 0:256].rearrange("p (h v) -> p h v", h=HPC)
                               for t in vt]
                        for k in range(NKT):
                            for gg in range(2):
                                st = 8 + 2 * w + gg
                                nc.tensor.matmul(
                                    vws[gg][:, :, :],
                                    xv_t[k][:, 128 * st:128 * st + 128],
                                    wv_t[k][:, :],
                                    start=(k == 0), stop=(k == NKT - 1))
                        for gg in range(2):
                            st = 8 + 2 * w + gg
                            nc.vector.tensor_copy(v2[:, st, 0:4:2, 0:64],
                                                  vws[gg][:, 0:4:2, :])
                            nc.vector.tensor_copy(v2[:, st, 1:4:2, 64:128],
                                                  vws[gg][:, 1:4:2, :])

                    # trip kinds: ("F", h, t) = full blocks 2t,2t+1 over the
                    # whole 512-q window; ("D1", h) = diag blocks 4j,4j+1;
                    # ("D2", h) = diag blocks 4j+2,4j+3.
                    def emit_scores(j, tr):
                        kind, h = tr[0], tr[1]
                        pair = h // 2
                        qb = 512 * j
                        stile = psB.tile([128, 1024], F32, name="stile",
                                         tag="stile", bufs=2)
                        if kind == "F":
                            t = tr[2]
                            # one accumulation group per PSUM bank
                            for n in range(2):
                                blk = 2 * t + n
                                nc.tensor.matmul(
                                    stile[:, 512 * n:512 * n + 512],
                                    kt[pair][:, 128 * blk:128 * blk + 128],
                                    qtz[h][:, qb:qb + 512],
                                    start=True, stop=True)
                        elif kind == "D1":
                            b0 = 4 * j
                            nc.tensor.matmul(
                                stile[:, 0:512],
                                kt[pair][:, 128 * b0:128 * b0 + 128],
                                qtz[h][:, qb:qb + 512], start=True, stop=True)
                            nc.tensor.matmul(
                                stile[:, 512:896],
                                kt[pair][:, 128 * (b0 + 1):128 * (b0 + 1) + 128],
                                qtz[h][:, qb + 128:qb + 512], start=True, stop=True)
                        else:  # D2
                            b0 = 4 * j
                            nc.tensor.matmul(
                                stile[:, 0:256],
                                kt[pair][:, 128 * (b0 + 2):128 * (b0 + 2) + 128],
                                qtz[h][:, qb + 256:qb + 512], start=True, stop=False)
                            nc.tensor.matmul(
                                stile[:, 256:384],
                                kt[pair][:, 128 * (b0 + 3):128 * (b0 + 3) + 128],
                                qtz[h][:, qb + 384:qb + 512], start=False, stop=True)
                        return stile

                    cm_v = cm_sb[:, :].rearrange("p (a b) -> p a b", a=2)

                    def emit_exp(j, tr, stile):
                        kind = tr[0]
                        w = {"F": 1024, "D1": 896, "D2": 384}[kind]
                        ptt = pb.tile([128, 1024], F32R, name="ptt", tag="ptt",
                                      bufs=3)
                        nc.scalar.activation(ptt[:, 0:w], stile[:, 0:w], AF.Exp)
                        if kind == "D1":
                            v = ptt[:, 0:1024].rearrange("p (a b) -> p a b",
                                                         a=2)[:, :, 0:128]
                            nc.vector.tensor_mul(v, v, cm_v)
                        elif kind == "D2":
                            v = ptt[:, 0:512].rearrange("p (a b) -> p a b",
                                                        a=2)[:, :, 0:128]
                            nc.vector.tensor_mul(v, v, cm_v)
                        return ptt

                    def emit_pv(j, tr, ptt, opsum, first):
                        kind, h = tr[0], tr[1]
                        if kind == "F":
                            t = tr[2]
                            for n in range(2):
                                blk = 2 * t + n
                                nc.tensor.matmul(
                                    opsum[:, 0:512],
                                    v2[:, blk, h, :],
                                    ptt[:, 512 * n:512 * n + 512],
                                    start=(first and n == 0), stop=False)
                        elif kind == "D1":
                            b0 = 4 * j
                            nc.tensor.matmul(
                                opsum[:, 0:512], v2[:, b0, h, :],
                                ptt[:, 0:512], start=first, stop=False)
                            nc.tensor.matmul(
                                opsum[:, 128:512], v2[:, b0 + 1, h, :],
                                ptt[:, 512:896], start=False, stop=False)
                        else:  # D2 -- always last in the head's accumulation
                            b0 = 4 * j
                            nc.tensor.matmul(
                                opsum[:, 256:512], v2[:, b0 + 2, h, :],
                                ptt[:, 0:256], start=False, stop=False)
                            nc.tensor.matmul(
                                opsum[:, 384:512], v2[:, b0 + 3, h, :],
                                ptt[:, 256:384], start=False, stop=True)

                    def emit_norm_den(h, opsum):
                        # den row -> SBUF right after the head's last PV; the
                        # PE-side broadcast is deferred one slot so this DVE
                        # copy's latency never stalls the PE queue.
                        drow = 64 if h % 2 == 0 else 0
                        den = pb.tile([128, 512], F32R, name="den", tag="den",
                                      bufs=2)
                        nc.vector.tensor_copy(den[drow:drow + 1, :],
                                              opsum[drow:drow + 1, :])
                        return den

                    def emit_norm_fin(j, h, opsum, den):
                        drow = 64 if h % 2 == 0 else 0
                        obase = 0 if h % 2 == 0 else 64
                        pair = h // 2
                        bcps = psB.tile([128, 512], F32, name="bcps", tag="ypsv",
                                        bufs=2)
                        nc.tensor.matmul(
                            bcps[:, :],
                            bcsel[drow:drow + 1, :],
                            den[drow:drow + 1, :],
                            start=True, stop=True)
                        rec = pb.tile([128, 512], F32, name="rec", tag="rec",
                                      bufs=1)
                        nc.vector.reciprocal_approx_fast(rec[:, :], bcps[:, :])
                        nc.vector.tensor_mul(
                            ot2[pair][obase:obase + 64, 512 * j:512 * j + 512],
                            opsum[obase:obase + 64, :],
                            rec[obase:obase + 64, :])

                    def emit_phase_c_group(jp, e):
                        yps = psB.tile([128, 512], F32, name="yps", tag="ypsv",
                                       bufs=2)
                        for p in range(2):
                            nc.tensor.matmul(
                                yps[:, :],
                                w0p[p][:, 128 * e:128 * e + 128],
                                ot2[p][:, 512 * jp:512 * jp + 512],
                                start=(p == 0), stop=(p == 1))
                        ysb = pb.tile([128, 512], FP16, name="ysb", tag="ysb",
                                      bufs=10)
                        nc.vector.tensor_copy(ysb[:, :], yps[:, :])
                        eng = nc.gpsimd if e % 2 == 0 else nc.sync
                        eng.dma_start(
                            out=yt[128 * e:128 * e + 128, 512 * jp:512 * jp + 512],
                            in_=ysb[:, :])

                    # jp=3 tail split: p=0 partial (pair 0) can run as soon
                    # as the pair-0 heads finish at j=3; only the small p=1
                    # add remains after the final norm.
                    y3p0 = {}

                    def emit_phase_c3_p0(e):
                        yps = psB.tile([128, 512], F32, name="y3ps", tag="ypsv",
                                       bufs=2)
                        nc.tensor.matmul(
                            yps[:, :],
                            w0p[0][:, 128 * e:128 * e + 128],
                            ot2[0][:, 1536:2048],
                            start=True, stop=True)
                        part = pb.tile([128, 512], FP16, name="y3sb", tag="ysb",
                                       bufs=10)
                        nc.vector.tensor_copy(part[:, :], yps[:, :])
                        y3p0[e] = part

                    def emit_phase_c3_p1(e):
                        yps = psB.tile([128, 512], F32, name="y3ps1", tag="ypsv",
                                       bufs=2)
                        nc.tensor.matmul(
                            yps[:, :],
                            w0p[1][:, 128 * e:128 * e + 128],
                            ot2[1][:, 1536:2048],
                            start=True, stop=True)
                        ysb = pb.tile([128, 512], FP16, name="ysb", tag="ysb",
                                      bufs=10)
                        nc.vector.tensor_add(ysb[:, :], yps[:, :], y3p0.pop(e)[:, :])
                        eng = nc.gpsimd if e % 2 == 0 else nc.sync
                        eng.dma_start(
                            out=yt[128 * e:128 * e + 128, 1536:2048],
                            in_=ysb[:, :])

                    vq = list(range(4))
                    for j in range(NJ):
                        trips = []
                        for h in range(HPC):
                            for t in range(2 * j):
                                trips.append(("F", h, t))
                            trips.append(("D1", h))
                            trips.append(("D2", h))
                        G = len(trips)
                        pcq = [(j - 1, e) for e in range(8)] if j > 0 else []
                        p0q = []
                        stiles, ptts, opsums = {}, {}, {}
                        norm_pend = None
                        for g in range(G + 3):
                            if j == 0 and vq and g < 4:
                                emit_v_wavelet(vq.pop(0))
                            # j<3: spread fillers every 3rd slot. j=3: hold
                            # them back to the late slots so the PE stays
                            # busy (HAM warm) right into the tail.
                            if j < NJ - 1:
                                if pcq and g >= 2 and g % 3 == 2:
                                    emit_phase_c_group(*pcq.pop(0))
                            else:
                                if pcq and g >= 15 and g % 2 == 1:
                                    emit_phase_c_group(*pcq.pop(0))
                            if p0q and g % 2 == 0:
                                emit_phase_c3_p0(p0q.pop(0))
                            if 2 <= g < G + 2:
                                tr = trips[g - 2]
                                h = tr[1]
                                first = (tr[0] == "D1" and j == 0) or \
                                    (tr[0] == "F" and tr[2] == 0)
                                if first:
                                    opsums[h] = psB.tile([128, 512], F32,
                                                         name="opsum", tag="acc",
                                                         bufs=2)
                                emit_pv(j, tr, ptts.pop(g - 2), opsums[h], first)
                            if g < G:
                                stiles[g] = emit_scores(j, trips[g])
                            # deferred by one slot: the den copy was issued
                            # last slot, so the bcps matmul won't stall the PE
                            if norm_pend is not None:
                                nh, nop, nden = norm_pend
                                emit_norm_fin(j, nh, nop, nden)
                                norm_pend = None
                                if j == NJ - 1 and nh == 1:
                                    p0q = list(range(8))
                            if 2 <= g < G + 2:
                                tr = trips[g - 2]
                                h = tr[1]
                                if tr[0] == "D2":
                                    den = emit_norm_den(h, opsums[h])
                                    norm_pend = (h, opsums.pop(h), den)
                            if 1 <= g <= G:
                                ptts[g - 1] = emit_exp(j, trips[g - 1],
                                                       stiles.pop(g - 1))
                        while pcq:
                            emit_phase_c_group(*pcq.pop(0))
                        while p0q:
                            emit_phase_c3_p0(p0q.pop(0))

                    for e in range(8):
                        emit_phase_c3_p1(e)

    nc.compile()
    return nc


def _run(inputs, trace=False):
    global _NC
    if _NC is None:
        _NC = _build()
    q = np.asarray(inputs["q"], dtype=np.float32)
    k = np.asarray(inputs["k"], dtype=np.float32)
    v = np.asarray(inputs["v"], dtype=np.float32)
    w_query = np.asarray(inputs["w_query"], dtype=np.float32)
    w_key = np.asarray(inputs["w_key"], dtype=np.float32)
    w_value = np.asarray(inputs["w_value"], dtype=np.float32)
    w_0 = np.asarray(inputs["w_0"], dtype=np.float32)

    tri = np.triu(np.ones((128, 128), dtype=np.float32))
    cmt = np.ascontiguousarray(np.tile(tri, (1, 2)))

    xq_b = [np.ascontiguousarray(q[b].T).astype(np.float16) for b in range(B)]
    xk_b = [np.ascontiguousarray(k[b].T).astype(np.float16) for b in range(B)]
    xv_b = [np.ascontiguousarray(v[b].T).astype(np.float16) for b in range(B)]

    in_maps = []
    for c in range(8):
        b, g = c // 4, c % 4
        sl = slice(256 * g, 256 * g + 256)
        in_maps.append({
            "xq": xq_b[b], "xk": xk_b[b], "xv": xv_b[b],
            "wq": np.ascontiguousarray(w_query[sl, :].T).astype(np.float16),
            "wk": np.ascontiguousarray(w_key[sl, :].T).astype(np.float16),
            "wv": np.ascontiguousarray(w_value[sl, :].T).astype(np.float16),
            "w0": np.ascontiguousarray(w_0[:, sl].T).astype(np.float16),
            "cmt": cmt,
        })

    res = run_bass_kernel_spmd(_NC, in_maps, core_ids=list(range(8)), trace=trace)
    y = np.empty((B, S, D), dtype=np.float32)
    for b in range(B):
        acc = res.results[4 * b]["yt"].astype(np.float32)
        for g in range(1, 4):
            acc += res.results[4 * b + g]["yt"].astype(np.float32)
        y[b] = acc.T
    return y, getattr(res, "exec_time_ns", None)


def kernel(**inputs):
    return _run(inputs, trace=False)[0]


# revision 26
# speedup vs baseline: 1.1958x; 1.1958x over previous
import sys

sys.path.insert(0, "/opt/trn_rl_repo")
import numpy as np
import concourse.bacc as bacc
import concourse.mybir as mybir
import concourse.tile as tile
from concourse.bass_utils import run_bass_kernel_spmd

F32R = mybir.dt.float32r
F32 = mybir.dt.float32
FP16 = mybir.dt.float16
AF = mybir.ActivationFunctionType

B, S, D, H, DV = 2, 2048, 1024, 16, 64
NKT = 8     # k-tiles of 128 over D
NJ = 4      # query chunks of 512
NB = 16     # key blocks of 128
HPC = 4     # heads per core

_NC = None


def _build():
    nc = bacc.Bacc(target_bir_lowering=False)
    xq = nc.dram_tensor("xq", [D, S], FP16, kind="ExternalInput")
    xk = nc.dram_tensor("xk", [D, S], FP16, kind="ExternalInput")
    xv = nc.dram_tensor("xv", [D, S], FP16, kind="ExternalInput")
    wq = nc.dram_tensor("wq", [D, 256], FP16, kind="ExternalInput")
    wk = nc.dram_tensor("wk", [D, 256], FP16, kind="ExternalInput")
    wv = nc.dram_tensor("wv", [D, 256], FP16, kind="ExternalInput")
    w0 = nc.dram_tensor("w0", [256, D], FP16, kind="ExternalInput")
    cmt = nc.dram_tensor("cmt", [128, 256], F32R, kind="ExternalInput")
    yt = nc.dram_tensor("yt", [D, S], FP16, kind="ExternalOutput")

    with tile.TileContext(nc) as tc:
        with tc.tile_pool(name="pp", bufs=1) as pp:
            # Per-head Q with the other head's 64 rows zeroed: scores use the
            # full dense 128-row kt block as stationary (full PE array) -- the
            # zero rows kill the other head's contribution.
            qtz = [pp.tile([128, S], FP16, name=f"qtz{h}", tag=f"qtz{h}")
                   for h in range(HPC)]
            kt = [pp.tile([128, S], FP16, name=f"kt{p}", tag=f"kt{p}") for p in range(2)]
            # V padded to 128 cols, all non-V columns = ones. Even heads keep
            # V in cols 0:64 (numerators -> out rows 0:64, den at row 64);
            # odd heads keep V in cols 64:128 (numerators -> out rows 64:128,
            # den read from row 0).
            v2 = pp.tile([128, NB, HPC, 128], F32R, name="v2", tag="v2")
            bcsel = pp.tile([128, 128], F32R, name="bcsel", tag="bcsel")
            w0p = [pp.tile([128, D], FP16, name=f"w0p{p}", tag=f"w0p{p}") for p in range(2)]
            ot2 = [pp.tile([128, S], FP16, name=f"ot2{p}", tag=f"ot2{p}") for p in range(2)]
            cm_sb = pp.tile([128, 256], F32R, name="cmsb", tag="cmsb")

            ones_stage = pp.tile([128, 512], F32, name="ones_stage", tag="ones_stage")
            nc.vector.memset(ones_stage[:, :], 1.0)
            for i in range(NB):
                nc.vector.tensor_copy(v2[:, i, :, :], ones_stage[:, :])
            nc.vector.tensor_copy(bcsel[64:65, :], ones_stage[64:65, 0:128])
            nc.vector.tensor_copy(bcsel[0:1, :], ones_stage[0:1, 0:128])
            for h in range(HPC):
                dead = 64 * (1 - (h % 2))
                nc.vector.memset(qtz[h][dead:dead + 64, :], 0.0)

            # xv/wv live through phase B (late V wavelets read them there)
            with tc.tile_pool(name="wtv", bufs=1) as wtv, \
                 tc.tile_pool(name="xvp", bufs=1) as xvp, \
                 tc.tile_pool(name="pb", bufs=1) as pb:
                wv_t = [wtv.tile([128, 256], FP16, name=f"wv{k}", tag=f"wv{k}")
                        for k in range(NKT)]
                xv_t = [xvp.tile([128, S], FP16, name=f"xv{k}", tag="xv", bufs=8)
                        for k in range(NKT)]
                for k in range(NKT):
                    nc.gpsimd.dma_start(out=wv_t[k][:, :],
                                        in_=wv[128 * k:128 * k + 128, :])
                for k in range(NKT):
                    eng = nc.sync if k % 2 == 0 else nc.gpsimd
                    eng.dma_start(out=xv_t[k][:, :], in_=xv[128 * k:128 * k + 128, :])

                # ---- Phase A: V first half, then Q, K projections ----
                with tc.tile_pool(name="wts", bufs=1) as wts, \
                     tc.tile_pool(name="xin", bufs=1) as xin, \
                     tc.tile_pool(name="psA", bufs=8, space="PSUM") as psA:
                    wq_t = [wts.tile([128, 256], FP16, name=f"wq{k}", tag=f"wq{k}")
                            for k in range(NKT)]
                    wk_t = [wts.tile([128, 256], FP16, name=f"wk{k}", tag=f"wk{k}")
                            for k in range(NKT)]
                    xq_t = [xin.tile([128, S], FP16, name=f"xq{k}", tag="xq", bufs=8)
                            for k in range(NKT)]
                    xk_t = [xin.tile([128, S], FP16, name=f"xk{k}", tag="xk", bufs=8)
                            for k in range(NKT)]
                    for k in range(NKT):
                        nc.scalar.dma_start(out=wq_t[k][:, :],
                                            in_=wq[128 * k:128 * k + 128, :])
                    for k in range(NKT):
                        eng = nc.sync if k % 2 == 0 else nc.scalar
                        eng.dma_start(out=xq_t[k][:, :], in_=xq[128 * k:128 * k + 128, :])
                    for k in range(NKT):
                        nc.scalar.dma_start(out=wk_t[k][:, :],
                                            in_=wk[128 * k:128 * k + 128, :])
                    for k in range(NKT):
                        eng = nc.sync if k % 2 == 0 else nc.scalar
                        eng.dma_start(out=xk_t[k][:, :], in_=xk[128 * k:128 * k + 128, :])
                    for p in range(2):
                        nc.sync.dma_start(out=w0p[p][:, :],
                                          in_=w0[128 * p:128 * p + 128, :])
                    nc.sync.dma_start(out=cm_sb[:, :], in_=cmt[:, :])

                    # V projection, first 8 st-groups (key blocks 0:8),
                    # k-outer so compute paces the xv DMA arrivals.
                    vps = [psA.tile([128, 4, 64], F32, name=f"vps{g}", tag="pj",
                                    bufs=8) for g in range(8)]
                    for k in range(NKT):
                        for g in range(8):
                            nc.tensor.matmul(
                                vps[g][:, :, :],
                                xv_t[k][:, 128 * g:128 * g + 128],
                                wv_t[k][:, :],
                                start=(k == 0), stop=(k == NKT - 1))
                    for g in range(8):
                        nc.vector.tensor_copy(v2[:, g, 0:4:2, 0:64],
                                              vps[g][:, 0:4:2, :])
                        nc.vector.tensor_copy(v2[:, g, 1:4:2, 64:128],
                                              vps[g][:, 1:4:2, :])

                    # Q then K: 8 psum groups [128,512] each, 512-wide
                    # matmuls, k-outer so compute paces DMA arrival.
                    for which, wt, xt in (("q", wq_t, xq_t), ("k", wk_t, xk_t)):
                        qps = [psA.tile([128, 512], F32, name=f"pj{which}{i}",
                                        tag="pj", bufs=8) for i in range(8)]
                        for k in range(NKT):
                            for p in range(2):
                                for jj in range(4):
                                    nc.tensor.matmul(
                                        qps[4 * p + jj][:, :],
                                        wt[k][:, 128 * p:128 * p + 128],
                                        xt[k][:, 512 * jj:512 * jj + 512],
                                        start=(k == 0), stop=(k == NKT - 1))
                        # q copies ride on DVE under the K matmul stream; the
                        # k copies gate the first scores, so they go jj-major
                        # and alternate DVE/ACT to land the jj=0 pair fastest.
                        if which == "q":
                            for p in range(2):
                                for jj in range(4):
                                    sl = slice(512 * jj, 512 * jj + 512)
                                    g = qps[4 * p + jj]
                                    nc.vector.tensor_copy(qtz[2 * p][0:64, sl],
                                                          g[0:64, :])
                                    nc.vector.tensor_copy(qtz[2 * p + 1][64:128, sl],
                                                          g[64:128, :])
                        else:
                            ci = 0
                            for jj in range(4):
                                for p in range(2):
                                    sl = slice(512 * jj, 512 * jj + 512)
                                    g = qps[4 * p + jj]
                                    if ci % 2 == 0:
                                        nc.vector.tensor_copy(kt[p][:, sl], g[:, :])
                                    else:
                                        nc.scalar.copy(kt[p][:, sl], g[:, :])
                                    ci += 1

                # ---- Phase B: attention + out-projection + late V half ----
                with tc.tile_pool(name="psB", bufs=1, space="PSUM") as psB:

                    def emit_v_wavelet(w):
                        # st-groups 8+2w, 9+2w; xv is fully resident by now.
                        vt = [psB.tile([128, 512], F32, name=f"vw{w}{gg}",
                                       tag="ypsv", bufs=2) for gg in range(2)]
                        vws = [t[:, 0:256].rearrange("p (h v) -> p h v", h=HPC)
                               for t in vt]
                        for k in range(NKT):
                            for gg in range(2):
                                st = 8 + 2 * w + gg
                                nc.tensor.matmul(
                                    vws[gg][:, :, :],
                                    xv_t[k][:, 128 * st:128 * st + 128],
                                    wv_t[k][:, :],
                                    start=(k == 0), stop=(k == NKT - 1))
                        for gg in range(2):
                            st = 8 + 2 * w + gg
                            nc.vector.tensor_copy(v2[:, st, 0:4:2, 0:64],
                                                  vws[gg][:, 0:4:2, :])
                            nc.vector.tensor_copy(v2[:, st, 1:4:2, 64:128],
                                                  vws[gg][:, 1:4:2, :])

                    # trip kinds: ("F", h, t) = full blocks 2t,2t+1 over the
                    # whole 512-q window; ("D1", h) = diag blocks 4j,4j+1;
                    # ("D2", h) = diag blocks 4j+2,4j+3.
                    def emit_scores(j, tr):
                        kind, h = tr[0], tr[1]
                        pair = h // 2
                        qb = 512 * j
                        stile = psB.tile([128, 1024], F32, name="stile",
                                         tag="stile", bufs=2)
                        if kind == "F":
                            t = tr[2]
                            # one accumulation group per PSUM bank
                            for n in range(2):
                                blk = 2 * t + n
                                nc.tensor.matmul(
                                    stile[:, 512 * n:512 * n + 512],
                                    kt[pair][:, 128 * blk:128 * blk + 128],
                                    qtz[h][:, qb:qb + 512],
                                    start=True, stop=True)
                        elif kind == "D1":
                            b0 = 4 * j
                            nc.tensor.matmul(
                                stile[:, 0:512],
                                kt[pair][:, 128 * b0:128 * b0 + 128],
                                qtz[h][:, qb:qb + 512], start=True, stop=True)
                            nc.tensor.matmul(
                                stile[:, 512:896],
                                kt[pair][:, 128 * (b0 + 1):128 * (b0 + 1) + 128],
                                qtz[h][:, qb + 128:qb + 512], start=True, stop=True)
                        else:  # D2
                            b0 = 4 * j
                            nc.tensor.matmul(
                                stile[:, 0:256],
                                kt[pair][:, 128 * (b0 + 2):128 * (b0 + 2) + 128],
                                qtz[h][:, qb + 256:qb + 512], start=True, stop=False)
                            nc.tensor.matmul(
                                stile[:, 256:384],
                                kt[pair][:, 128 * (b0 + 3):128 * (b0 + 3) + 128],
                                qtz[h][:, qb + 384:qb + 512], start=False, stop=True)
                        return stile

                    cm_v = cm_sb[:, :].rearrange("p (a b) -> p a b", a=2)

                    def emit_exp(j, tr, stile):
                        kind = tr[0]
                        w = {"F": 1024, "D1": 896, "D2": 384}[kind]
                        ptt = pb.tile([128, 1024], F32R, name="ptt", tag="ptt",
                                      bufs=3)
                        nc.scalar.activation(ptt[:, 0:w], stile[:, 0:w], AF.Exp)
                        if kind == "D1":
                            v = ptt[:, 0:1024].rearrange("p (a b) -> p a b",
                                                         a=2)[:, :, 0:128]
                            nc.vector.tensor_mul(v, v, cm_v)
                        elif kind == "D2":
                            v = ptt[:, 0:512].rearrange("p (a b) -> p a b",
                                                        a=2)[:, :, 0:128]
                            nc.vector.tensor_mul(v, v, cm_v)
                        return ptt

                    def emit_pv(j, tr, ptt, opsum, first):
                        kind, h = tr[0], tr[1]
                        if kind == "F":
                            t = tr[2]
                            for n in range(2):
                                blk = 2 * t + n
                                nc.tensor.matmul(
                                    opsum[:, 0:512],
                                    v2[:, blk, h, :],
                                    ptt[:, 512 * n:512 * n + 512],
                                    start=(first and n == 0), stop=False)
                        elif kind == "D1":
                            b0 = 4 * j
                            nc.tensor.matmul(
                                opsum[:, 0:512], v2[:, b0, h, :],
                                ptt[:, 0:512], start=first, stop=False)
                            nc.tensor.matmul(
                                opsum[:, 128:512], v2[:, b0 + 1, h, :],
                                ptt[:, 512:896], start=False, stop=False)
                        else:  # D2 -- always last in the head's accumulation
                            b0 = 4 * j
                            nc.tensor.matmul(
                                opsum[:, 256:512], v2[:, b0 + 2, h, :],
                                ptt[:, 0:256], start=False, stop=False)
                            nc.tensor.matmul(
                                opsum[:, 384:512], v2[:, b0 + 3, h, :],
                                ptt[:, 256:384], start=False, stop=True)

                    def emit_norm_den(h, opsum):
                        # den row -> SBUF right after the head's last PV; the
                        # PE-side broadcast is deferred one slot so this DVE
                        # copy's latency never stalls the PE queue.
                        drow = 64 if h % 2 == 0 else 0
                        den = pb.tile([128, 512], F32R, name="den", tag="den",
                                      bufs=2)
                        nc.vector.tensor_copy(den[drow:drow + 1, :],
                                              opsum[drow:drow + 1, :])
                        return den

                    def emit_norm_fin(j, h, opsum, den):
                        drow = 64 if h % 2 == 0 else 0
                        obase = 0 if h % 2 == 0 else 64
                        pair = h // 2
                        bcps = psB.tile([128, 512], F32, name="bcps", tag="ypsv",
                                        bufs=2)
                        nc.tensor.matmul(
                            bcps[:, :],
                            bcsel[drow:drow + 1, :],
                            den[drow:drow + 1, :],
                            start=True, stop=True)
                        rec = pb.tile([128, 512], F32, name="rec", tag="rec",
                                      bufs=1)
                        nc.vector.reciprocal_approx_fast(rec[:, :], bcps[:, :])
                        nc.vector.tensor_mul(
                            ot2[pair][obase:obase + 64, 512 * j:512 * j + 512],
                            opsum[obase:obase + 64, :],
                            rec[obase:obase + 64, :])

                    def emit_phase_c_group(jp, e):
                        yps = psB.tile([128, 512], F32, name="yps", tag="ypsv",
                                       bufs=2)
                        for p in range(2):
                            nc.tensor.matmul(
                                yps[:, :],
                                w0p[p][:, 128 * e:128 * e + 128],
                                ot2[p][:, 512 * jp:512 * jp + 512],
                                start=(p == 0), stop=(p == 1))
                        ysb = pb.tile([128, 512], FP16, name="ysb", tag="ysb",
                                      bufs=10)
                        nc.vector.tensor_copy(ysb[:, :], yps[:, :])
                        eng = nc.gpsimd if e % 2 == 0 else nc.sync
                        eng.dma_start(
                            out=yt[128 * e:128 * e + 128, 512 * jp:512 * jp + 512],
                            in_=ysb[:, :])

                    # jp=3 tail split: p=0 partial (pair 0) can run as soon
                    # as the pair-0 heads finish at j=3; only the small p=1
                    # add remains after the final norm.
                    y3p0 = {}

                    def emit_phase_c3_p0(e):
                        yps = psB.tile([128, 512], F32, name="y3ps", tag="ypsv",
                                       bufs=2)
                        nc.tensor.matmul(
                            yps[:, :],
                            w0p[0][:, 128 * e:128 * e + 128],
                            ot2[0][:, 1536:2048],
                            start=True, stop=True)
                        part = pb.tile([128, 512], FP16, name="y3sb", tag="ysb",
                                       bufs=10)
                        nc.vector.tensor_copy(part[:, :], yps[:, :])
                        y3p0[e] = part

                    def emit_phase_c3_p1(e):
                        yps = psB.tile([128, 512], F32, name="y3ps1", tag="ypsv",
                                       bufs=2)
                        nc.tensor.matmul(
                            yps[:, :],
                            w0p[1][:, 128 * e:128 * e + 128],
                            ot2[1][:, 1536:2048],
                            start=True, stop=True)
                        ysb = pb.tile([128, 512], FP16, name="ysb", tag="ysb",
                                      bufs=10)
                        nc.vector.tensor_add(ysb[:, :], yps[:, :], y3p0.pop(e)[:, :])
                        eng = nc.gpsimd if e % 2 == 0 else nc.sync
                        eng.dma_start(
                            out=yt[128 * e:128 * e + 128, 1536:2048],
                            in_=ysb[:, :])

                    vq = list(range(4))
                    for j in range(NJ):
                        trips = []
                        for h in range(HPC):
                            for t in range(2 * j):
                                trips.append(("F", h, t))
                            trips.append(("D1", h))
                            trips.append(("D2", h))
                        G = len(trips)
                        pcq = [(j - 1, e) for e in range(8)] if j > 0 else []
                        p0q = []
                        stiles, ptts, opsums = {}, {}, {}
                        norm_pend = None
                        for g in range(G + 3):
                            if j == 0 and vq and g < 4:
                                emit_v_wavelet(vq.pop(0))
                            if pcq and g >= 2 and g % 3 == 2:
                                emit_phase_c_group(*pcq.pop(0))
                            elif p0q and g % 2 == 0:
                                emit_phase_c3_p0(p0q.pop(0))
                            # scores first in the slot: the ACT engine's
                            # exp(g) is gated on them, while pv(g-2) gates
                            # nothing until the next slot.
                            if g < G:
                                stiles[g] = emit_scores(j, trips[g])
                            if 2 <= g < G + 2:
                                tr = trips[g - 2]
                                h = tr[1]
                                first = (tr[0] == "D1" and j == 0) or \
                                    (tr[0] == "F" and tr[2] == 0)
                                if first:
                                    opsums[h] = psB.tile([128, 512], F32,
                                                         name="opsum", tag="acc",
                                                         bufs=2)
                                emit_pv(j, tr, ptts.pop(g - 2), opsums[h], first)
                            # deferred by one slot: the den copy was issued
                            # last slot, so the bcps matmul won't stall the PE
                            if norm_pend is not None:
                                nh, nop, nden = norm_pend
                                emit_norm_fin(j, nh, nop, nden)
                                norm_pend = None
                                if j == NJ - 1 and nh == 1:
                                    p0q = list(range(8))
                            if 2 <= g < G + 2:
                                tr = trips[g - 2]
                                h = tr[1]
                                if tr[0] == "D2":
                                    den = emit_norm_den(h, opsums[h])
                                    norm_pend = (h, opsums.pop(h), den)
                            if 1 <= g <= G:
                                ptts[g - 1] = emit_exp(j, trips[g - 1],
                                                       stiles.pop(g - 1))
                        while pcq:
                            emit_phase_c_group(*pcq.pop(0))
                        while p0q:
                            emit_phase_c3_p0(p0q.pop(0))

                    for e in range(8):
                        emit_phase_c3_p1(e)

    nc.compile()
    return nc


def _run(inputs, trace=False):
    global _NC
    if _NC is None:
        _NC = _build()
    q = np.asarray(inputs["q"], dtype=np.float32)
    k = np.asarray(inputs["k"], dtype=np.float32)
    v = np.asarray(inputs["v"], dtype=np.float32)
    w_query = np.asarray(inputs["w_query"], dtype=np.float32)
    w_key = np.asarray(inputs["w_key"], dtype=np.float32)
    w_value = np.asarray(inputs["w_value"], dtype=np.float32)
    w_0 = np.asarray(inputs["w_0"], dtype=np.float32)

    tri = np.triu(np.ones((128, 128), dtype=np.float32))
    cmt = np.ascontiguousarray(np.tile(tri, (1, 2)))

    xq_b = [np.ascontiguousarray(q[b].T).astype(np.float16) for b in range(B)]
    xk_b = [np.ascontiguousarray(k[b].T).astype(np.float16) for b in range(B)]
    xv_b = [np.ascontiguousarray(v[b].T).astype(np.float16) for b in range(B)]

    in_maps = []
    for c in range(8):
        b, g = c // 4, c % 4
        sl = slice(256 * g, 256 * g + 256)
        in_maps.append({
            "xq": xq_b[b], "xk": xk_b[b], "xv": xv_b[b],
            "wq": np.ascontiguousarray(w_query[sl, :].T).astype(np.float16),
            "wk": np.ascontiguousarray(w_key[sl, :].T).astype(np.float16),
            "wv": np.ascontiguousarray(w_value[sl, :].T).astype(np.float16),
            "w0": np.ascontiguousarray(w_0[:, sl].T).astype(np.float16),
            "cmt": cmt,
        })

    res = run_bass_kernel_spmd(_NC, in_maps, core_ids=list(range(8)), trace=trace)
    y = np.empty((B, S, D), dtype=np.float32)
    for b in range(B):
        acc = res.results[4 * b]["yt"].astype(np.float32)
        for g in range(1, 4):
            acc += res.results[4 * b + g]["yt"].astype(np.float32)
        y[b] = acc.T
    return y, getattr(res, "exec_time_ns", None)


def kernel(**inputs):
    return _run(inputs, trace=False)[0]


# revision 27
# speedup vs baseline: 1.2082x; 1.0104x over previous
import sys

sys.path.insert(0, "/opt/trn_rl_repo")
import numpy as np
import concourse.bacc as bacc
import concourse.mybir as mybir
import concourse.tile as tile
from concourse.bass_utils import run_bass_kernel_spmd

F32R = mybir.dt.float32r
F32 = mybir.dt.float32
FP16 = mybir.dt.float16
AF = mybir.ActivationFunctionType

B, S, D, H, DV = 2, 2048, 1024, 16, 64
NKT = 8     # k-tiles of 128 over D
NJ = 4      # query chunks of 512
NB = 16     # key blocks of 128
HPC = 4     # heads per core

_NC = None


def _build():
    nc = bacc.Bacc(target_bir_lowering=False)
    xq = nc.dram_tensor("xq", [D, S], FP16, kind="ExternalInput")
    xk = nc.dram_tensor("xk", [D, S], FP16, kind="ExternalInput")
    xv = nc.dram_tensor("xv", [D, S], FP16, kind="ExternalInput")
    wq = nc.dram_tensor("wq", [D, 256], FP16, kind="ExternalInput")
    wk = nc.dram_tensor("wk", [D, 256], FP16, kind="ExternalInput")
    wv = nc.dram_tensor("wv", [D, 256], FP16, kind="ExternalInput")
    w0 = nc.dram_tensor("w0", [256, D], FP16, kind="ExternalInput")
    cmt = nc.dram_tensor("cmt", [128, 256], F32R, kind="ExternalInput")
    yt = nc.dram_tensor("yt", [D, S], FP16, kind="ExternalOutput")

    with tile.TileContext(nc) as tc:
        with tc.tile_pool(name="pp", bufs=1) as pp:
            # Per-head Q with the other head's 64 rows zeroed: scores use the
            # full dense 128-row kt block as stationary (full PE array) -- the
            # zero rows kill the other head's contribution.
            qtz = [pp.tile([128, S], FP16, name=f"qtz{h}", tag=f"qtz{h}")
                   for h in range(HPC)]
            kt = [pp.tile([128, S], FP16, name=f"kt{p}", tag=f"kt{p}") for p in range(2)]
            # V padded to 128 cols, all non-V columns = ones. Even heads keep
            # V in cols 0:64 (numerators -> out rows 0:64, den at row 64);
            # odd heads keep V in cols 64:128 (numerators -> out rows 64:128,
            # den read from row 0).
            v2 = pp.tile([128, NB, HPC, 128], F32R, name="v2", tag="v2")
            bcsel = pp.tile([128, 128], F32R, name="bcsel", tag="bcsel")
            w0p = [pp.tile([128, D], FP16, name=f"w0p{p}", tag=f"w0p{p}") for p in range(2)]
            ot2 = [pp.tile([128, S], FP16, name=f"ot2{p}", tag=f"ot2{p}") for p in range(2)]
            cm_sb = pp.tile([128, 256], F32R, name="cmsb", tag="cmsb")

            ones_stage = pp.tile([128, 512], F32, name="ones_stage", tag="ones_stage")
            nc.vector.memset(ones_stage[:, :], 1.0)
            for i in range(NB):
                nc.vector.tensor_copy(v2[:, i, :, :], ones_stage[:, :])
            nc.vector.tensor_copy(bcsel[64:65, :], ones_stage[64:65, 0:128])
            nc.vector.tensor_copy(bcsel[0:1, :], ones_stage[0:1, 0:128])
            for h in range(HPC):
                dead = 64 * (1 - (h % 2))
                nc.vector.memset(qtz[h][dead:dead + 64, :], 0.0)

            # xv/wv live through phase B (late V wavelets read them there)
            with tc.tile_pool(name="wtv", bufs=1) as wtv, \
                 tc.tile_pool(name="xvp", bufs=1) as xvp, \
                 tc.tile_pool(name="pb", bufs=1) as pb:
                wv_t = [wtv.tile([128, 256], FP16, name=f"wv{k}", tag=f"wv{k}")
                        for k in range(NKT)]
                xv_t = [xvp.tile([128, S], FP16, name=f"xv{k}", tag="xv", bufs=8)
                        for k in range(NKT)]
                for k in range(NKT):
                    nc.gpsimd.dma_start(out=wv_t[k][:, :],
                                        in_=wv[128 * k:128 * k + 128, :])
                for k in range(NKT):
                    eng = nc.sync if k % 2 == 0 else nc.gpsimd
                    eng.dma_start(out=xv_t[k][:, :], in_=xv[128 * k:128 * k + 128, :])

                # ---- Phase A: V first half, then Q, K projections ----
                with tc.tile_pool(name="wts", bufs=1) as wts, \
                     tc.tile_pool(name="xin", bufs=1) as xin, \
                     tc.tile_pool(name="psA", bufs=8, space="PSUM") as psA:
                    wq_t = [wts.tile([128, 256], FP16, name=f"wq{k}", tag=f"wq{k}")
                            for k in range(NKT)]
                    wk_t = [wts.tile([128, 256], FP16, name=f"wk{k}", tag=f"wk{k}")
                            for k in range(NKT)]
                    xq_t = [xin.tile([128, S], FP16, name=f"xq{k}", tag="xq", bufs=8)
                            for k in range(NKT)]
                    xk_t = [xin.tile([128, S], FP16, name=f"xk{k}", tag="xk", bufs=8)
                            for k in range(NKT)]
                    for k in range(NKT):
                        nc.scalar.dma_start(out=wq_t[k][:, :],
                                            in_=wq[128 * k:128 * k + 128, :])
                    for k in range(NKT):
                        eng = nc.sync if k % 2 == 0 else nc.scalar
                        eng.dma_start(out=xq_t[k][:, :], in_=xq[128 * k:128 * k + 128, :])
                    for k in range(NKT):
                        nc.scalar.dma_start(out=wk_t[k][:, :],
                                            in_=wk[128 * k:128 * k + 128, :])
                    for k in range(NKT):
                        eng = nc.sync if k % 2 == 0 else nc.scalar
                        eng.dma_start(out=xk_t[k][:, :], in_=xk[128 * k:128 * k + 128, :])
                    for p in range(2):
                        nc.sync.dma_start(out=w0p[p][:, :],
                                          in_=w0[128 * p:128 * p + 128, :])
                    nc.sync.dma_start(out=cm_sb[:, :], in_=cmt[:, :])

                    # V projection, first 8 st-groups (key blocks 0:8),
                    # k-outer so compute paces the xv DMA arrivals.
                    vps = [psA.tile([128, 4, 64], F32, name=f"vps{g}", tag="pj",
                                    bufs=8) for g in range(8)]
                    for k in range(NKT):
                        for g in range(8):
                            nc.tensor.matmul(
                                vps[g][:, :, :],
                                xv_t[k][:, 128 * g:128 * g + 128],
                                wv_t[k][:, :],
                                start=(k == 0), stop=(k == NKT - 1))
                    for g in range(8):
                        nc.vector.tensor_copy(v2[:, g, 0:4:2, 0:64],
                                              vps[g][:, 0:4:2, :])
                        nc.vector.tensor_copy(v2[:, g, 1:4:2, 64:128],
                                              vps[g][:, 1:4:2, :])

                    # Q then K: 8 psum groups [128,512] each, 512-wide
                    # matmuls, k-outer so compute paces DMA arrival.
                    for which, wt, xt in (("q", wq_t, xq_t), ("k", wk_t, xk_t)):
                        qps = [psA.tile([128, 512], F32, name=f"pj{which}{i}",
                                        tag="pj", bufs=8) for i in range(8)]
                        for k in range(NKT):
                            for p in range(2):
                                for jj in range(4):
                                    nc.tensor.matmul(
                                        qps[4 * p + jj][:, :],
                                        wt[k][:, 128 * p:128 * p + 128],
                                        xt[k][:, 512 * jj:512 * jj + 512],
                                        start=(k == 0), stop=(k == NKT - 1))
                        # q copies ride on DVE under the K matmul stream; the
                        # k copies gate the first scores, so they go jj-major
                        # and alternate DVE/ACT to land the jj=0 pair fastest.
                        if which == "q":
                            for p in range(2):
                                for jj in range(4):
                                    sl = slice(512 * jj, 512 * jj + 512)
                                    g = qps[4 * p + jj]
                                    nc.vector.tensor_copy(qtz[2 * p][0:64, sl],
                                                          g[0:64, :])
                                    nc.vector.tensor_copy(qtz[2 * p + 1][64:128, sl],
                                                          g[64:128, :])
                        else:
                            ci = 0
                            for jj in range(4):
                                for p in range(2):
                                    sl = slice(512 * jj, 512 * jj + 512)
                                    g = qps[4 * p + jj]
                                    if ci % 2 == 0:
                                        nc.vector.tensor_copy(kt[p][:, sl], g[:, :])
                                    else:
                                        nc.scalar.copy(kt[p][:, sl], g[:, :])
                                    ci += 1

                # ---- Phase B: attention + out-projection + late V half ----
                with tc.tile_pool(name="psB", bufs=1, space="PSUM") as psB:

                    def emit_v_wavelet(w):
                        # st-groups 8+2w, 9+2w; xv is fully resident by now.
                        vt = [psB.tile([128, 512], F32, name=f"vw{w}{gg}",
                                       tag="ypsv", bufs=2) for gg in range(2)]
                        vws = [t[:, 0:256].rearrange("p (h v) -> p h v", h=HPC)
                               for t in vt]
                        for k in range(NKT):
                            for gg in range(2):
                                st = 8 + 2 * w + gg
                                nc.tensor.matmul(
                                    vws[gg][:, :, :],
                                    xv_t[k][:, 128 * st:128 * st + 128],
                                    wv_t[k][:, :],
                                    start=(k == 0), stop=(k == NKT - 1))
                        for gg in range(2):
                            st = 8 + 2 * w + gg
                            nc.vector.tensor_copy(v2[:, st, 0:4:2, 0:64],
                                                  vws[gg][:, 0:4:2, :])
                            nc.vector.tensor_copy(v2[:, st, 1:4:2, 64:128],
                                                  vws[gg][:, 1:4:2, :])

                    # trip kinds: ("F", h, t) = full blocks 2t,2t+1 over the
                    # whole 512-q window; ("D1", h) = diag blocks 4j,4j+1;
                    # ("D2", h) = diag blocks 4j+2,4j+3.
                    def emit_scores(j, tr):
                        kind, h = tr[0], tr[1]
                        pair = h // 2
                        qb = 512 * j
                        stile = psB.tile([128, 1024], F32, name="stile",
                                         tag="stile", bufs=2)
                        if kind == "F":
                            t = tr[2]
                            # one accumulation group per PSUM bank
                            for n in range(2):
                                blk = 2 * t + n
                                nc.tensor.matmul(
                                    stile[:, 512 * n:512 * n + 512],
                                    kt[pair][:, 128 * blk:128 * blk + 128],
                                    qtz[h][:, qb:qb + 512],
                                    start=True, stop=True)
                        elif kind == "D1":
                            b0 = 4 * j
                            nc.tensor.matmul(
                                stile[:, 0:512],
                                kt[pair][:, 128 * b0:128 * b0 + 128],
                                qtz[h][:, qb:qb + 512], start=True, stop=True)
                            nc.tensor.matmul(
                                stile[:, 512:896],
                                kt[pair][:, 128 * (b0 + 1):128 * (b0 + 1) + 128],
                                qtz[h][:, qb + 128:qb + 512], start=True, stop=True)
                        else:  # D2
                            b0 = 4 * j
                            nc.tensor.matmul(
                                stile[:, 0:256],
                                kt[pair][:, 128 * (b0 + 2):128 * (b0 + 2) + 128],
                                qtz[h][:, qb + 256:qb + 512], start=True, stop=False)
                            nc.tensor.matmul(
                                stile[:, 256:384],
                                kt[pair][:, 128 * (b0 + 3):128 * (b0 + 3) + 128],
                                qtz[h][:, qb + 384:qb + 512], start=False, stop=True)
                        return stile

                    cm_v = cm_sb[:, :].rearrange("p (a b) -> p a b", a=2)

                    def emit_exp(j, tr, stile):
                        kind = tr[0]
                        w = {"F": 1024, "D1": 896, "D2": 384}[kind]
                        ptt = pb.tile([128, 1024], F32R, name="ptt", tag="ptt",
                                      bufs=3)
                        nc.scalar.activation(ptt[:, 0:w], stile[:, 0:w], AF.Exp)
                        if kind == "D1":
                            v = ptt[:, 0:1024].rearrange("p (a b) -> p a b",
                                                         a=2)[:, :, 0:128]
                            nc.vector.tensor_mul(v, v, cm_v)
                        elif kind == "D2":
                            v = ptt[:, 0:512].rearrange("p (a b) -> p a b",
                                                        a=2)[:, :, 0:128]
                            nc.vector.tensor_mul(v, v, cm_v)
                        return ptt

                    def emit_pv(j, tr, ptt, opsum, first):
                        kind, h = tr[0], tr[1]
                        if kind == "F":
                            t = tr[2]
                            for n in range(2):
                                blk = 2 * t + n
                                nc.tensor.matmul(
                                    opsum[:, 0:512],
                                    v2[:, blk, h, :],
                                    ptt[:, 512 * n:512 * n + 512],
                                    start=(first and n == 0), stop=False)
                        elif kind == "D1":
                            b0 = 4 * j
                            nc.tensor.matmul(
                                opsum[:, 0:512], v2[:, b0, h, :],
                                ptt[:, 0:512], start=first, stop=False)
                            nc.tensor.matmul(
                                opsum[:, 128:512], v2[:, b0 + 1, h, :],
                                ptt[:, 512:896], start=False, stop=False)
                        else:  # D2 -- always last in the head's accumulation
                            b0 = 4 * j
                            nc.tensor.matmul(
                                opsum[:, 256:512], v2[:, b0 + 2, h, :],
                                ptt[:, 0:256], start=False, stop=False)
                            nc.tensor.matmul(
                                opsum[:, 384:512], v2[:, b0 + 3, h, :],
                                ptt[:, 256:384], start=False, stop=True)

                    def emit_norm_den(h, opsum):
                        # den row -> SBUF right after the head's last PV; the
                        # PE-side broadcast is deferred one slot so this DVE
                        # copy's latency never stalls the PE queue.
                        drow = 64 if h % 2 == 0 else 0
                        den = pb.tile([128, 512], F32R, name="den", tag="den",
                                      bufs=2)
                        nc.vector.tensor_copy(den[drow:drow + 1, :],
                                              opsum[drow:drow + 1, :])
                        return den

                    def emit_norm_fin(j, h, opsum, den):
                        drow = 64 if h % 2 == 0 else 0
                        obase = 0 if h % 2 == 0 else 64
                        pair = h // 2
                        bcps = psB.tile([128, 512], F32, name="bcps", tag="ypsv",
                                        bufs=2)
                        nc.tensor.matmul(
                            bcps[:, :],
                            bcsel[drow:drow + 1, :],
                            den[drow:drow + 1, :],
                            start=True, stop=True)
                        rec = pb.tile([128, 512], F32, name="rec", tag="rec",
                                      bufs=1)
                        nc.vector.reciprocal_approx_fast(rec[:, :], bcps[:, :])
                        nc.vector.tensor_mul(
                            ot2[pair][obase:obase + 64, 512 * j:512 * j + 512],
                            opsum[obase:obase + 64, :],
                            rec[obase:obase + 64, :])

                    def emit_phase_c_group(jp, e):
                        yps = psB.tile([128, 512], F32, name="yps", tag="ypsv",
                                       bufs=2)
                        for p in range(2):
                            nc.tensor.matmul(
                                yps[:, :],
                                w0p[p][:, 128 * e:128 * e + 128],
                                ot2[p][:, 512 * jp:512 * jp + 512],
                                start=(p == 0), stop=(p == 1))
                        ysb = pb.tile([128, 512], FP16, name="ysb", tag="ysb",
                                      bufs=10)
                        nc.vector.tensor_copy(ysb[:, :], yps[:, :])
                        eng = nc.gpsimd if e % 2 == 0 else nc.sync
                        eng.dma_start(
                            out=yt[128 * e:128 * e + 128, 512 * jp:512 * jp + 512],
                            in_=ysb[:, :])

                    # jp=3 tail split: p=0 partial (pair 0) can run as soon
                    # as the pair-0 heads finish at j=3; only the small p=1
                    # add remains after the final norm.
                    y3p0 = {}

                    def emit_phase_c3_p0(e):
                        yps = psB.tile([128, 512], F32, name="y3ps", tag="ypsv",
                                       bufs=2)
                        nc.tensor.matmul(
                            yps[:, :],
                            w0p[0][:, 128 * e:128 * e + 128],
                            ot2[0][:, 1536:2048],
                            start=True, stop=True)
                        part = pb.tile([128, 512], FP16, name="y3sb", tag="ysb",
                                       bufs=10)
                        nc.vector.tensor_copy(part[:, :], yps[:, :])
                        y3p0[e] = part

                    def emit_phase_c3_p1(e):
                        yps = psB.tile([128, 512], F32, name="y3ps1", tag="ypsv",
                                       bufs=2)
                        nc.tensor.matmul(
                            yps[:, :],
                            w0p[1][:, 128 * e:128 * e + 128],
                            ot2[1][:, 1536:2048],
                            start=True, stop=True)
                        ysb = pb.tile([128, 512], FP16, name="ysb", tag="ysb",
                                      bufs=10)
                        nc.vector.tensor_add(ysb[:, :], yps[:, :], y3p0.pop(e)[:, :])
                        eng = nc.gpsimd if e % 2 == 0 else nc.sync
                        eng.dma_start(
                            out=yt[128 * e:128 * e + 128, 1536:2048],
                            in_=ysb[:, :])

                    vq = list(range(4))
                    for j in range(NJ):
                        trips = []
                        for h in range(HPC):
                            for t in range(2 * j):
                                trips.append(("F", h, t))
                            trips.append(("D1", h))
                            trips.append(("D2", h))
                        G = len(trips)
                        pcq = [(j - 1, e) for e in range(8)] if j > 0 else []
                        p0q = []
                        stiles, ptts, opsums = {}, {}, {}
                        norm_pend = None
                        for g in range(G + 3):
                            if j == 0 and vq and g < 4:
                                emit_v_wavelet(vq.pop(0))
                            if pcq and g >= 2 and g % 3 == 2:
                                emit_phase_c_group(*pcq.pop(0))
                            elif p0q and g % 2 == 0:
                                emit_phase_c3_p0(p0q.pop(0))
                            if 2 <= g < G + 2:
                                tr = trips[g - 2]
                                h = tr[1]
                                first = (tr[0] == "D1" and j == 0) or \
                                    (tr[0] == "F" and tr[2] == 0)
                                if first:
                                    opsums[h] = psB.tile([128, 512], F32,
                                                         name="opsum", tag="acc",
                                                         bufs=2)
                                emit_pv(j, tr, ptts.pop(g - 2), opsums[h], first)
                            if g < G:
                                stiles[g] = emit_scores(j, trips[g])
                            # deferred by one slot: the den copy was issued
                            # last slot, so the bcps matmul won't stall the PE
                            if norm_pend is not None:
                                nh, nop, nden = norm_pend
                                emit_norm_fin(j, nh, nop, nden)
                                norm_pend = None
                                if j == NJ - 1 and nh == 1:
                                    p0q = list(range(8))
                            if 2 <= g < G + 2:
                                tr = trips[g - 2]
                                h = tr[1]
                                if tr[0] == "D2":
                                    den = emit_norm_den(h, opsums[h])
                                    norm_pend = (h, opsums.pop(h), den)
                            if 1 <= g <= G:
                                ptts[g - 1] = emit_exp(j, trips[g - 1],
                                                       stiles.pop(g - 1))
                        while pcq:
                            emit_phase_c_group(*pcq.pop(0))
                        while p0q:
                            emit_phase_c3_p0(p0q.pop(0))

                    for e in range(8):
                        emit_phase_c3_p1(e)

    nc.compile()
    return nc


def _run(inputs, trace=False):
    global _NC
    if _NC is None:
        _NC = _build()
    q = np.asarray(inputs["q"], dtype=np.float32)
    k = np.asarray(inputs["k"], dtype=np.float32)
    v = np.asarray(inputs["v"], dtype=np.float32)
    w_query = np.asarray(inputs["w_query"], dtype=np.float32)
    w_key = np.asarray(inputs["w_key"], dtype=np.float32)
    w_value = np.asarray(inputs["w_value"], dtype=np.float32)
    w_0 = np.asarray(inputs["w_0"], dtype=np.float32)

    tri = np.triu(np.ones((128, 128), dtype=np.float32))
    cmt = np.ascontiguousarray(np.tile(tri, (1, 2)))

    xq_b = [np.ascontiguousarray(q[b].T).astype(np.float16) for b in range(B)]
    xk_b = [np.ascontiguousarray(k[b].T).astype(np.float16) for b in range(B)]
    xv_b = [np.ascontiguousarray(v[b].T).astype(np.float16) for b in range(B)]

    in_maps = []
    for c in range(8):
        b, g = c // 4, c % 4
        sl = slice(256 * g, 256 * g + 256)
        in_maps.append({
            "xq": xq_b[b], "xk": xk_b[b], "xv": xv_b[b],
            "wq": np.ascontiguousarray(w_query[sl, :].T).astype(np.float16),
            "wk": np.ascontiguousarray(w_key[sl, :].T).astype(np.float16),
            "wv": np.ascontiguousarray(w_value[sl, :].T).astype(np.float16),
            "w0": np.ascontiguousarray(w_0[:, sl].T).astype(np.float16),
            "cmt": cmt,
        })

    res = run_bass_kernel_spmd(_NC, in_maps, core_ids=list(range(8)), trace=trace)
    y = np.empty((B, S, D), dtype=np.float32)
    for b in range(B):
        acc = res.results[4 * b]["yt"].astype(np.float32)
        for g in range(1, 4):
            acc += res.results[4 * b + g]["yt"].astype(np.float32)
        y[b] = acc.T
    return y, getattr(res, "exec_time_ns", None)


def kernel(**inputs):
    return _run(inputs, trace=False)[0]
